# revision 32
# baseline (speedup 1.0000x reference)
"""DSFourierAttention Trainium2 kernel (v4).

Math (per (b, h) slice, validated vs the jax reference):
    qf = rfft(q, ortho) etc. as dense DFT matmuls (Fre/Fim [L, 512])
    qk_T[y, x] = sum_e (kfr qfr + kfi qfi)     (stacked [re; im] K=128 matmul)
    im_T[y, x] = sum_e (-kfi qfr + kfr qfi)    (kswp = [-kfi; kfr])
    p = exp(sqrt(re^2 + im^2))                 (no max subtraction; |qk| <= ~5)
    qkv_T[x, e] = (p^T @ [vfr | vfi | ones]) / colsum   (ones col -> colsum)
    out[l, e] = Gre^T @ qkvr + Gim^T @ qkvi    (irfft weights w = [1, 2...])
    out = out * tau[b] + delta[b, l]

Numerics: the x=512 (Nyquist) OUTPUT bin is dropped (~2e-3 extra rel err)
but the y=512 softmax term is kept. The Nyquist COSINE column is smuggled
through Fim's column 0 (normally all-zero: -sin(0)); the polluted bin-0
imag entries are memset back to 0 after stacking, and v's Nyquist row is
recovered from the vf imag tile's partition 0 before it is zeroed.

QK re/im go into ONE 2-bank PSUM tile -> a single ACT square pass
[128,1024] -> DVE add of the two bf16 halves.

Sharding: batch-parallel, 2 batches per core across 8 cores.
"""

import os
import sys

import numpy as np

for _p in ("/opt/trn_rl_repo", "/root/.axon_site/_ro/trn_rl_repo"):
    if os.path.isdir(_p) and _p not in sys.path:
        sys.path.insert(0, _p)

import ml_dtypes  # noqa: E402
import concourse.bass as bass  # noqa: E402
import concourse.tile as tile  # noqa: E402
from concourse import bacc, mybir  # noqa: E402
from concourse.bass_utils import run_bass_kernel_spmd  # noqa: E402

B, L, H, E = 16, 1024, 8, 64
X = 512                 # main frequency bins; Nyquist handled as rank-1 extras
NCORES = 8
BL = B // NCORES        # 2 batches per core
NLC = L // 128          # 8 l-chunks
NYC = 4                 # 128-row y chunks
NXC = 4                 # 128-row x chunks

F32 = mybir.dt.float32
BF16 = mybir.dt.bfloat16
AF = mybir.ActivationFunctionType

LAST_RESULT = None


def _consts():
    l = np.arange(L)
    xs = np.arange(X)
    ang = 2.0 * np.pi * np.outer(l, xs) / L          # [L, X]
    fre = (np.cos(ang) / np.sqrt(L)).astype(ml_dtypes.bfloat16)
    fim = (-np.sin(ang) / np.sqrt(L)).astype(ml_dtypes.bfloat16)
    # smuggle the Nyquist cosine column through fim's dead bin-0 column
    fim[:, 0] = (np.cos(np.pi * l) / np.sqrt(L)).astype(ml_dtypes.bfloat16)
    w = np.full(X, 2.0)
    w[0] = 1.0
    gre = (w[:, None] * np.cos(ang.T) / np.sqrt(L)).astype(ml_dtypes.bfloat16)
    gim = (w[:, None] * -np.sin(ang.T) / np.sqrt(L)).astype(ml_dtypes.bfloat16)
    return fre, fim, gre, gim


def build_module(bl=BL, compile=True):
    from concourse.alu_op_type import AluOpType

    nc = bacc.Bacc("TRN2", target_bir_lowering=False, debug=False,
                   num_devices=NCORES)

    qd = nc.dram_tensor("qd", [bl, L, H, E], BF16, kind="ExternalInput").ap()
    kd = nc.dram_tensor("kd", [bl, L, H, E], BF16, kind="ExternalInput").ap()
    vd = nc.dram_tensor("vd", [bl, L, H, E], BF16, kind="ExternalInput").ap()
    taud = nc.dram_tensor("taud", [bl, 1], F32, kind="ExternalInput").ap()
    deltad = nc.dram_tensor("deltad", [bl, L], F32, kind="ExternalInput").ap()
    fred = nc.dram_tensor("fred", [L, X], BF16, kind="ExternalInput").ap()
    fimd = nc.dram_tensor("fimd", [L, X], BF16, kind="ExternalInput").ap()
    gred = nc.dram_tensor("gred", [X, L], BF16, kind="ExternalInput").ap()
    gimd = nc.dram_tensor("gimd", [X, L], BF16, kind="ExternalInput").ap()
    outd = nc.dram_tensor("outd", [bl, L, H, E], BF16, kind="ExternalOutput").ap()

    with tile.TileContext(nc) as tc:
        _body(nc, tc, AluOpType, qd, kd, vd, taud, deltad, fred, fimd,
              gred, gimd, outd, bl)
    if compile:
        nc.compile()
    return nc


class P:
    """Pool/tile handles shared across phases."""


def _body(nc, tc, OPS, qd, kd, vd, taud, deltad, fred, fimd,
          gred, gimd, outd, bl=BL):
    from contextlib import ExitStack

    ctx = ExitStack()
    with ctx:
        st = P()
        st.OPS = OPS
        consts = ctx.enter_context(tc.tile_pool(name="consts", bufs=1))
        st.io = ctx.enter_context(tc.tile_pool(name="io", bufs=2))
        st.stg = ctx.enter_context(tc.tile_pool(name="stg", bufs=2))
        st.stk = ctx.enter_context(tc.tile_pool(name="stk", bufs=8))
        st.sq = ctx.enter_context(tc.tile_pool(name="sq", bufs=3))
        st.sp = ctx.enter_context(tc.tile_pool(name="sp", bufs=4))
        st.pp = ctx.enter_context(tc.tile_pool(name="pp", bufs=4))
        st.rag = ctx.enter_context(tc.tile_pool(name="rag", bufs=2))
        st.vfp = ctx.enter_context(tc.tile_pool(name="vfp", bufs=2))
        st.qkvp = ctx.enter_context(tc.tile_pool(name="qkvp", bufs=8))
        st.ep = ctx.enter_context(tc.tile_pool(name="ep", bufs=2))
        st.ps = ctx.enter_context(tc.tile_pool(name="ps", bufs=4, space="PSUM"))
        st.pq = ctx.enter_context(tc.tile_pool(name="pq", bufs=2, space="PSUM"))

        # ---- constants -------------------------------------------------
        st.fre = consts.tile([128, NLC, X], BF16, name="fre_sb")
        st.fim = consts.tile([128, NLC, X], BF16, name="fim_sb")
        nc.sync.dma_start(out=st.fre[:, :, :],
                          in_=fred.rearrange("(c p) x -> p c x", p=128))
        nc.sync.dma_start(out=st.fim[:, :, :],
                          in_=fimd.rearrange("(c p) x -> p c x", p=128))
        st.gre = consts.tile([128, NXC, L], BF16, name="gre_sb")
        st.gim = consts.tile([128, NXC, L], BF16, name="gim_sb")

        st.qd, st.kd, st.vd = qd, kd, vd
        st.taud, st.deltad, st.outd = taud, deltad, outd

        vf = _phase_v(nc, st, 0)
        pending = None          # deferred iFFT of the previous batch
        stkA = None             # wave-0 FFT, possibly pre-emitted last batch
        for b in range(bl):
            tau_sb = st.ep.tile([128, 1], F32, tag="tau", name=f"tau{b}")
            nc.sync.dma_start(out=tau_sb[:, :],
                              in_=taud[b:b + 1, 0:1].to_broadcast([128, 1]))
            delta_sb = st.ep.tile([128, NLC], F32, tag="delta", name=f"delta{b}")
            nc.sync.dma_start(out=delta_sb[:, :],
                              in_=deltad[b, :].rearrange("(c p) -> p c", p=128))
            qkv_all = [st.qkvp.tile([128, 2, H, 64], BF16, tag="qkv",
                                    name=f"qkv{b}_{xc}") for xc in range(NXC)]

            if stkA is None:
                stkA = {}
                for hp in (0, 1):
                    stkA.update(_fft_qk(nc, st, b, hp))
            if b == 0:
                # iFFT weights are first needed ~100us in; don't let their
                # 2 MiB of DMA delay the critical q/k/v lead-in loads
                nc.sync.dma_start(out=st.gre[:, :, :],
                                  in_=gred.rearrange("(c p) l -> p c l", p=128))
                nc.sync.dma_start(out=st.gim[:, :, :],
                                  in_=gimd.rearrange("(c p) l -> p c l", p=128))
            have_next = b + 1 < bl
            last = not have_next

            stkB_box = {}

            def fill_fftB(bb=b):
                for hp in (2, 3):
                    stkB_box.update(_fft_qk(nc, st, bb, hp))

            def fill_next(bb=b):
                # next batch's v-FFT and wave-0 q/k FFT fill the wave-1
                # sqrt/exp sections of this batch
                st.vf_next = _phase_v(nc, st, bb + 1)
                nxt = {}
                for hp in (0, 1):
                    nxt.update(_fft_qk(nc, st, bb + 1, hp))
                st.stkA_next = nxt

            def ichunk(lo, hi, p=pending):
                def go():
                    _ifft(nc, st, *p, range(lo, hi))
                return go

            st.vf_next = None
            st.stkA_next = None
            # wave-1 FFT fills wave-0's sqrt/exp sections; deferred
            # prev-batch iFFT chunks and next-batch v-FFT/q-k-FFT fill the
            # wave-1 sections and the final-batch tail.
            sqs0 = _wave_qk(nc, st, b, 0, stkA)
            _wave_sqrt(nc, st, sqs0)
            fill_fftB()
            ps0 = _wave_exp(nc, st, sqs0)
            if last and pending is not None:
                ichunk(0, 2)()
            _wave_av(nc, st, b, 0, ps0, vf, qkv_all)
            sqs1 = _wave_qk(nc, st, b, 1, stkB_box)
            _wave_sqrt(nc, st, sqs1)
            if last and pending is not None:
                ichunk(2, 4)()
            elif have_next:
                fill_next()
            elif pending is not None:
                _ifft(nc, st, *pending, range(0, NLC))
                pending = None
            ps1 = _wave_exp(nc, st, sqs1)
            if last and pending is not None:
                ichunk(4, NLC)()
                pending = None
            _wave_av(nc, st, b, 1, ps1, vf, qkv_all)
            stkA = st.stkA_next
            if have_next:
                pending = (b, tau_sb, delta_sb, qkv_all)
                vf = st.vf_next
            else:
                _ifft(nc, st, b, tau_sb, delta_sb, qkv_all, range(0, NLC), act_epi=True)


def _ifft(nc, st, b, tau_sb, delta_sb, qkv_all, lcs_range, act_epi=False):
    for lc in lcs_range:
        lcs = slice(lc * 128, (lc + 1) * 128)
        ps_o = st.ps.tile([128, 512], F32, tag="ps", name=f"pso{b}_{lc}")
        for xc in range(NXC):
            nc.tensor.matmul(ps_o[:, 0:512],
                             st.gre[:, xc, lcs],
                             qkv_all[xc][:, 0, :, :].rearrange("p h e -> p (h e)"),
                             start=(xc == 0), stop=False)
            nc.tensor.matmul(ps_o[:, 0:512],
                             st.gim[:, xc, lcs],
                             qkv_all[xc][:, 1, :, :].rearrange("p h e -> p (h e)"),
                             start=False, stop=(xc == NXC - 1))
        out_t = st.ep.tile([128, 512], BF16, tag="outt", bufs=3,
                           name=f"out{b}_{lc}")
        if act_epi:
            # Identity(scale*x + bias) = tau*x + delta; Identity is in
            # every ACT table set, so this cannot thrash table loads
            nc.scalar.activation(out=out_t[:, :], in_=ps_o[:, 0:512],
                                 func=AF.Identity,
                                 bias=delta_sb[:, lc:lc + 1],
                                 scale=tau_sb[:, 0:1])
        else:
            nc.vector.tensor_scalar(out=out_t[:, :], in0=ps_o[:, 0:512],
                                    scalar1=tau_sb[:, 0:1],
                                    scalar2=delta_sb[:, lc:lc + 1],
                                    op0=st.OPS.mult, op1=st.OPS.add)
        nc.sync.dma_start(
            out=st.outd[b, lcs, :, :].rearrange("l h e -> l (h e)"),
            in_=out_t[:, :])


def _phase_v(nc, st, b):
    """v[b] load + transposed FFT -> (vf_av[yc], v512) in AV layout."""
    v_sb = st.io.tile([128, NLC, H * E], BF16, tag="vsb", bufs=2,
                      name=f"vsb{b}")
    nc.sync.dma_start(
        out=v_sb[:, :, :],
        in_=st.vd[b].rearrange("(c p) h e -> p c (h e)", p=128))

    vf_av = []
    for yc in range(NYC):
        t = st.vfp.tile([128, H, 132], BF16, tag="vfav", bufs=8,
                        name=f"vfav{b}_{yc}")
        vf_av.append(t)
    v512 = st.vfp.tile([1, H, 132], BF16, tag="v512", bufs=2, name=f"v512_{b}")

    for part, f_sb in ((0, st.fre), (1, st.fim)):
        for yc in range(NYC):
            ps = st.ps.tile([128, 512], F32, tag="ps", name=f"psv{b}_{part}_{yc}")
            for c in range(NLC):
                nc.tensor.matmul(ps[:, 0:512],
                                 f_sb[:, c, yc * 128:(yc + 1) * 128],
                                 v_sb[:, c, :],
                                 start=(c == 0), stop=(c == NLC - 1))
            nc.vector.tensor_copy(
                out=vf_av[yc][:, :, part * 64:(part + 1) * 64],
                in_=ps[:, 0:512].rearrange("p (h e) -> p h e", h=H))
    # vf_av[0] imag partition 0 actually holds vf Nyquist RE (smuggled
    # through fim col 0): save it as v512's real part, then zero it.
    nc.gpsimd.dma_start(out=v512[0:1, :, 0:64],
                        in_=vf_av[0][0:1, :, 64:128])
    nc.vector.memset(vf_av[0][0:1, :, 64:128], 0.0)
    for yc in range(NYC):
        nc.vector.memset(vf_av[yc][:, :, 128:129], 1.0)
    nc.vector.memset(v512[0:1, :, 64:128], 0.0)
    nc.vector.memset(v512[0:1, :, 128:129], 1.0)
    return vf_av, v512


def _fft_qk(nc, st, b, hp):
    """FFT of q & k for head pair hp -> stacked qstk/kstk/kswp/krag tiles."""
    q_hp = st.io.tile([128, NLC, 128], BF16, tag="qhp", bufs=2,
                      name=f"qhp{b}_{hp}")
    nc.sync.dma_start(
        out=q_hp[:, :, :],
        in_=st.qd[b, :, 2 * hp:2 * hp + 2, :].rearrange("(c p) h e -> p c (h e)", p=128))
    k_hp = st.io.tile([128, NLC, 128], BF16, tag="khp", bufs=2,
                      name=f"khp{b}_{hp}")
    nc.sync.dma_start(
        out=k_hp[:, :, :],
        in_=st.kd[b, :, 2 * hp:2 * hp + 2, :].rearrange("(c p) h e -> p c (h e)", p=128))

    out = {}
    ps_qr = st.ps.tile([128, 512], F32, tag="ps", name=f"psqr{b}_{hp}")
    ps_qi = st.ps.tile([128, 512], F32, tag="ps", name=f"psqi{b}_{hp}")
    for c in range(NLC):
        lhsT = q_hp[:, c, :]
        nc.tensor.matmul(ps_qr[:, 0:512], lhsT, st.fre[:, c, :],
                         start=(c == 0), stop=(c == NLC - 1))
        nc.tensor.matmul(ps_qi[:, 0:512], lhsT, st.fim[:, c, :],
                         start=(c == 0), stop=(c == NLC - 1))
    st_qr = st.stg.tile([128, X], BF16, tag="st", bufs=6, name=f"stqr{b}_{hp}")
    nc.vector.tensor_scalar_mul(out=st_qr[:, :], in0=ps_qr[:, 0:512],
                                scalar1=0.125)
    st_qi = st.stg.tile([128, X], BF16, tag="st", bufs=6, name=f"stqi{b}_{hp}")
    nc.vector.tensor_scalar_mul(out=st_qi[:, :], in0=ps_qi[:, 0:512],
                                scalar1=0.125)

    ps_kr = st.ps.tile([128, 512], F32, tag="ps", name=f"pskr{b}_{hp}")
    ps_ki = st.ps.tile([128, 512], F32, tag="ps", name=f"pski{b}_{hp}")
    for c in range(NLC):
        lhsT = k_hp[:, c, :]
        nc.tensor.matmul(ps_kr[:, 0:512], lhsT, st.fre[:, c, :],
                         start=(c == 0), stop=(c == NLC - 1))
        nc.tensor.matmul(ps_ki[:, 0:512], lhsT, st.fim[:, c, :],
                         start=(c == 0), stop=(c == NLC - 1))
    st_kr = st.stg.tile([128, X], BF16, tag="st", bufs=6, name=f"stkr{b}_{hp}")
    nc.vector.tensor_copy(out=st_kr[:, :], in_=ps_kr[:, 0:512])
    st_ki = st.stg.tile([128, X], BF16, tag="st", bufs=6, name=f"stki{b}_{hp}")
    nc.vector.tensor_copy(out=st_ki[:, :], in_=ps_ki[:, 0:512])
    st_kin = st.stg.tile([128, X], BF16, tag="st", bufs=6, name=f"stkin{b}_{hp}")
    nc.vector.tensor_scalar_mul(out=st_kin[:, :], in0=ps_ki[:, 0:512],
                                scalar1=-1.0)

    for phi in range(2):
        h = 2 * hp + phi
        rows = slice(64 * phi, 64 * phi + 64)
        # st_qi/st_ki col 0 hold the head's Nyquist RE value (smuggled);
        # zero the stacked copies' bin-0 imag after the row DMAs.
        qstk = st.stk.tile([128, X], BF16, tag="qstk", name=f"qstk{b}_{h}")
        nc.sync.dma_start(out=qstk[0:64, :], in_=st_qr[rows, :])
        nc.sync.dma_start(out=qstk[64:128, :], in_=st_qi[rows, :])
        nc.gpsimd.memset(qstk[64:128, 0:1], 0.0)
        kstk = st.stk.tile([128, X], BF16, tag="kstk", name=f"kstk{b}_{h}")
        nc.sync.dma_start(out=kstk[0:64, :], in_=st_kr[rows, :])
        nc.sync.dma_start(out=kstk[64:128, :], in_=st_ki[rows, :])
        nc.gpsimd.memset(kstk[64:128, 0:1], 0.0)
        kswp = st.stk.tile([128, X], BF16, tag="kswp", name=f"kswp{b}_{h}")
        nc.sync.dma_start(out=kswp[0:64, :], in_=st_kin[rows, :])
        nc.sync.dma_start(out=kswp[64:128, :], in_=st_kr[rows, :])
        nc.gpsimd.memset(kswp[0:64, 0:1], 0.0)
        krag = st.stk.tile([128, 2], BF16, tag="krag", name=f"krag{b}_{h}")
        nc.gpsimd.memset(krag[:, :], 0.0)
        nc.gpsimd.dma_start(out=krag[0:64, 0:1], in_=st_ki[rows, 0:1])
        nc.gpsimd.dma_start(out=krag[64:128, 1:2], in_=st_ki[rows, 0:1])
        out[h] = (qstk, kstk, kswp, krag)
    return out


def _qk_head(nc, st, b, h, stk, ragS, ragI, i):
    """QK matmuls + |qk|^2 squares for one head; rag row i of ragS/ragI."""
    qstk, kstk, kswp, krag = stk[h]
    s_h = st.sp.tile([128, NYC, X], F32, tag="s", name=f"s{b}_{h}")
    # ragged y=512 attention row first so the rag chain (squares, add,
    # sqrt, exp) completes early — AV's rank-1 matmul needs it
    ps_g = st.ps.tile([128, 512], F32, tag="ps", name=f"psg{b}_{h}")
    nc.tensor.matmul(ps_g[0:2, 0:512], krag[:, :], qstk[:, 0:512],
                     start=True, stop=True)
    rg2 = st.rag.tile([2, X], BF16, tag="rg2", name=f"rg2{b}_{h}")
    nc.vector.tensor_copy(out=rg2[:, :], in_=ps_g[0:2, 0:512])
    nc.gpsimd.dma_start(out=ragS[i:i + 1, :], in_=rg2[0:1, :])
    nc.gpsimd.dma_start(out=ragI[i:i + 1, :], in_=rg2[1:2, :])
    for yc in range(NYC):
        ycs = slice(yc * 128, (yc + 1) * 128)
        pq = st.pq.tile([128, 1024], F32, tag="pq", name=f"pq{b}_{h}_{yc}")
        nc.tensor.matmul(pq[:, 0:512], kstk[:, ycs], qstk[:, 0:512],
                         start=True, stop=True)
        nc.tensor.matmul(pq[:, 512:1024], kswp[:, ycs], qstk[:, 0:512],
                         start=True, stop=True)
        tsq = st.sq.tile([128, 1024], BF16, tag="tsq", name=f"tsq{b}_{h}_{yc}")
        nc.scalar.square(out=tsq[:, :], in_=pq[:, :])
        nc.vector.tensor_add(s_h[:, yc, :], tsq[:, 0:512], tsq[:, 512:1024])
    return s_h


def _av_head(nc, st, b, h, p_h, er_h, vf, qkv_all):
    vf_av, v512 = vf
    for xc in range(NXC):
        xcs = slice(xc * 128, (xc + 1) * 128)
        ps_av = st.ps.tile([128, 512], F32, tag="ps", name=f"psav{b}_{h}_{xc}")
        for yc in range(NYC):
            nc.tensor.matmul(ps_av[:, 0:129], p_h[:, yc, xcs],
                             vf_av[yc][:, h, 0:129],
                             start=(yc == 0), stop=False)
        # rank-1 y=512 term last so the group never waits on the rag chain
        nc.tensor.matmul(ps_av[:, 0:129], er_h[0:1, xcs],
                         v512[0:1, h, 0:129], start=False, stop=True)
        rc = st.ep.tile([128, 1], F32, tag="rc", bufs=4, name=f"rc{b}_{h}_{xc}")
        nc.vector.reciprocal(out=rc[:, :], in_=ps_av[:, 128:129])
        nc.vector.tensor_scalar_mul(
            out=qkv_all[xc][:, :, h, :],
            in0=ps_av[:, 0:128].rearrange("p (t e) -> p t e", t=2),
            scalar1=rc[:, 0:1])


def _wave_qk(nc, st, b, w, stk):
    """QK + squares for a wave's 4 heads, plus the rag chain up to ragM."""
    heads = [4 * w + i for i in range(4)]
    ragS = st.rag.tile([4, X], BF16, tag="ragS", name=f"ragS{b}_{w}")
    ragI = st.rag.tile([4, X], BF16, tag="ragI", name=f"ragI{b}_{w}")
    s_tiles = {}
    for i, h in enumerate(heads):
        s_tiles[h] = _qk_head(nc, st, b, h, stk, ragS, ragI, i)
    # rag chain on fast engines (DVE add): if its sqrt became ready only
    # mid-exp-section, the scheduler would thrash table loads
    nc.scalar.square(out=ragS[:, :], in_=ragS[:, :])
    nc.scalar.square(out=ragI[:, :], in_=ragI[:, :])
    ragM = st.rag.tile([4, X], F32, tag="ragM", name=f"ragM{b}_{w}")
    nc.vector.tensor_add(ragM[:, :], ragS[:, :], ragI[:, :])
    return heads, s_tiles, ragM


def _wave_sqrt(nc, st, sqs):
    heads, s_tiles, ragM = sqs
    nc.scalar.sqrt(out=ragM[:, :], in_=ragM[:, :])
    for h in heads:
        nc.scalar.sqrt(out=s_tiles[h][:, :, :], in_=s_tiles[h][:, :, :])


def _wave_exp(nc, st, sqs):
    heads, s_tiles, ragM = sqs
    b_w = heads[0] // 4
    # rag exp first so the er copies land early
    ragP = st.rag.tile([4, X], BF16, tag="ragP", name=f"ragP_{b_w}")
    nc.scalar.activation(out=ragP[:, :], in_=ragM[:, :], func=AF.Exp)
    er = {}
    for i, h in enumerate(heads):
        e_h = st.rag.tile([1, X], BF16, tag="er", bufs=6, name=f"er_{h}")
        nc.gpsimd.dma_start(out=e_h[0:1, :], in_=ragP[i:i + 1, :])
        er[h] = e_h
    p_tiles = {}
    for h in heads:
        p_h = st.pp.tile([128, NYC, X], BF16, tag="p", name=f"p_{h}")
        nc.scalar.activation(out=p_h[:, :, :], in_=s_tiles[h][:, :, :],
                             func=AF.Exp)
        p_tiles[h] = p_h
    return p_tiles, er


def _wave_av(nc, st, b, w, ps_in, vf, qkv_all):
    heads = [4 * w + i for i in range(4)]
    p_tiles, er = ps_in
    for h in heads:
        _av_head(nc, st, b, h, p_tiles[h], er[h], vf, qkv_all)


_BUILT = None
_CONSTS = None


def _get_built():
    global _BUILT, _CONSTS
    if _BUILT is None:
        _BUILT = build_module()
        _CONSTS = _consts()
    return _BUILT, _CONSTS


def kernel(q, k, v, mask, tau, delta):
    global LAST_RESULT
    nc, (fre, fim, gre, gim) = _get_built()
    q = np.ascontiguousarray(np.asarray(q, dtype=np.float32)).astype(ml_dtypes.bfloat16)
    k = np.ascontiguousarray(np.asarray(k, dtype=np.float32)).astype(ml_dtypes.bfloat16)
    v = np.ascontiguousarray(np.asarray(v, dtype=np.float32)).astype(ml_dtypes.bfloat16)
    tau = np.ascontiguousarray(np.asarray(tau, dtype=np.float32))
    delta = np.ascontiguousarray(np.asarray(delta, dtype=np.float32))

    in_maps = []
    for i in range(NCORES):
        sl = slice(i * BL, (i + 1) * BL)
        in_maps.append({
            "qd": np.ascontiguousarray(q[sl]),
            "kd": np.ascontiguousarray(k[sl]),
            "vd": np.ascontiguousarray(v[sl]),
            "taud": np.ascontiguousarray(tau[sl]),
            "deltad": np.ascontiguousarray(delta[sl]),
            "fred": fre, "fimd": fim,
            "gred": gre, "gimd": gim,
        })
    res = run_bass_kernel_spmd(nc, in_maps, core_ids=list(range(NCORES)))
    LAST_RESULT = res
    out = np.concatenate([res.results[i]["outd"] for i in range(NCORES)], axis=0)
    return out.astype(np.float32)


# revision 33
# speedup vs baseline: 1.1599x; 1.1599x over previous
"""DSFourierAttention Trainium2 kernel (v4).

Math (per (b, h) slice, validated vs the jax reference):
    qf = rfft(q, ortho) etc. as dense DFT matmuls (Fre/Fim [L, 512])
    qk_T[y, x] = sum_e (kfr qfr + kfi qfi)     (stacked [re; im] K=128 matmul)
    im_T[y, x] = sum_e (-kfi qfr + kfr qfi)    (kswp = [-kfi; kfr])
    p = exp(sqrt(re^2 + im^2))                 (no max subtraction; |qk| <= ~5)
    qkv_T[x, e] = (p^T @ [vfr | vfi | ones]) / colsum   (ones col -> colsum)
    out[l, e] = Gre^T @ qkvr + Gim^T @ qkvi    (irfft weights w = [1, 2...])
    out = out * tau[b] + delta[b, l]

Numerics: the x=512 (Nyquist) OUTPUT bin is dropped (~2e-3 extra rel err)
but the y=512 softmax term is kept. The Nyquist COSINE column is smuggled
through Fim's column 0 (normally all-zero: -sin(0)); the polluted bin-0
imag entries are memset back to 0 after stacking, and v's Nyquist row is
recovered from the vf imag tile's partition 0 before it is zeroed.

QK re/im go into ONE 2-bank PSUM tile -> a single ACT square pass
[128,1024] -> DVE add of the two bf16 halves.

Sharding: batch-parallel, 2 batches per core across 8 cores.
"""

import os
import sys

import numpy as np

for _p in ("/opt/trn_rl_repo", "/root/.axon_site/_ro/trn_rl_repo"):
    if os.path.isdir(_p) and _p not in sys.path:
        sys.path.insert(0, _p)

import ml_dtypes  # noqa: E402
import concourse.bass as bass  # noqa: E402
import concourse.tile as tile  # noqa: E402
from concourse import bacc, mybir  # noqa: E402
from concourse.bass_utils import run_bass_kernel_spmd  # noqa: E402

B, L, H, E = 16, 1024, 8, 64
X = 512                 # main frequency bins; Nyquist handled as rank-1 extras
NCORES = 8
BL = B // NCORES        # 2 batches per core
NLC = L // 128          # 8 l-chunks
NYC = 4                 # 128-row y chunks
NXC = 4                 # 128-row x chunks

F32 = mybir.dt.float32
BF16 = mybir.dt.bfloat16
AF = mybir.ActivationFunctionType

LAST_RESULT = None


def _consts():
    l = np.arange(L)
    xs = np.arange(X)
    ang = 2.0 * np.pi * np.outer(l, xs) / L          # [L, X]
    fre = (np.cos(ang) / np.sqrt(L)).astype(ml_dtypes.bfloat16)
    fim = (-np.sin(ang) / np.sqrt(L)).astype(ml_dtypes.bfloat16)
    # smuggle the Nyquist cosine column through fim's dead bin-0 column
    fim[:, 0] = (np.cos(np.pi * l) / np.sqrt(L)).astype(ml_dtypes.bfloat16)
    w = np.full(X, 2.0)
    w[0] = 1.0
    gre = (w[:, None] * np.cos(ang.T) / np.sqrt(L)).astype(ml_dtypes.bfloat16)
    gim = (w[:, None] * -np.sin(ang.T) / np.sqrt(L)).astype(ml_dtypes.bfloat16)
    return fre, fim, gre, gim


def build_module(bl=BL, compile=True):
    from concourse.alu_op_type import AluOpType

    nc = bacc.Bacc("TRN2", target_bir_lowering=False, debug=False,
                   num_devices=NCORES)

    qd = nc.dram_tensor("qd", [bl, L, H, E], BF16, kind="ExternalInput").ap()
    kd = nc.dram_tensor("kd", [bl, L, H, E], BF16, kind="ExternalInput").ap()
    vd = nc.dram_tensor("vd", [bl, L, H, E], BF16, kind="ExternalInput").ap()
    taud = nc.dram_tensor("taud", [bl, 1], F32, kind="ExternalInput").ap()
    deltad = nc.dram_tensor("deltad", [bl, L], F32, kind="ExternalInput").ap()
    fred = nc.dram_tensor("fred", [L, X], BF16, kind="ExternalInput").ap()
    fimd = nc.dram_tensor("fimd", [L, X], BF16, kind="ExternalInput").ap()
    gred = nc.dram_tensor("gred", [X, L], BF16, kind="ExternalInput").ap()
    gimd = nc.dram_tensor("gimd", [X, L], BF16, kind="ExternalInput").ap()
    outd = nc.dram_tensor("outd", [bl, L, H, E], BF16, kind="ExternalOutput").ap()

    with tile.TileContext(nc) as tc:
        _body(nc, tc, AluOpType, qd, kd, vd, taud, deltad, fred, fimd,
              gred, gimd, outd, bl)
    if compile:
        nc.compile()
    return nc


class P:
    """Pool/tile handles shared across phases."""


def _body(nc, tc, OPS, qd, kd, vd, taud, deltad, fred, fimd,
          gred, gimd, outd, bl=BL):
    from contextlib import ExitStack

    ctx = ExitStack()
    with ctx:
        st = P()
        st.OPS = OPS
        consts = ctx.enter_context(tc.tile_pool(name="consts", bufs=1))
        st.io = ctx.enter_context(tc.tile_pool(name="io", bufs=2))
        st.stg = ctx.enter_context(tc.tile_pool(name="stg", bufs=2))
        st.stk = ctx.enter_context(tc.tile_pool(name="stk", bufs=8))
        st.sq = ctx.enter_context(tc.tile_pool(name="sq", bufs=3))
        st.sp = ctx.enter_context(tc.tile_pool(name="sp", bufs=4))
        st.pp = ctx.enter_context(tc.tile_pool(name="pp", bufs=4))
        st.rag = ctx.enter_context(tc.tile_pool(name="rag", bufs=2))
        st.vfp = ctx.enter_context(tc.tile_pool(name="vfp", bufs=2))
        st.qkvp = ctx.enter_context(tc.tile_pool(name="qkvp", bufs=8))
        st.ep = ctx.enter_context(tc.tile_pool(name="ep", bufs=2))
        st.ps = ctx.enter_context(tc.tile_pool(name="ps", bufs=4, space="PSUM"))
        st.pq = ctx.enter_context(tc.tile_pool(name="pq", bufs=2, space="PSUM"))

        # ---- constants -------------------------------------------------
        st.fre = consts.tile([128, NLC, X], BF16, name="fre_sb")
        st.fim = consts.tile([128, NLC, X], BF16, name="fim_sb")
        nc.sync.dma_start(out=st.fre[:, :, :],
                          in_=fred.rearrange("(c p) x -> p c x", p=128))
        nc.sync.dma_start(out=st.fim[:, :, :],
                          in_=fimd.rearrange("(c p) x -> p c x", p=128))
        st.gre = consts.tile([128, NXC, L], BF16, name="gre_sb")
        st.gim = consts.tile([128, NXC, L], BF16, name="gim_sb")

        st.qd, st.kd, st.vd = qd, kd, vd
        st.taud, st.deltad, st.outd = taud, deltad, outd

        vf = _phase_v(nc, st, 0)
        pending = None          # deferred iFFT of the previous batch
        stkA = None             # wave-0 FFT, possibly pre-emitted last batch
        for b in range(bl):
            tau_sb = st.ep.tile([128, 1], F32, tag="tau", name=f"tau{b}")
            nc.sync.dma_start(out=tau_sb[:, :],
                              in_=taud[b:b + 1, 0:1].to_broadcast([128, 1]))
            delta_sb = st.ep.tile([128, NLC], F32, tag="delta", name=f"delta{b}")
            nc.sync.dma_start(out=delta_sb[:, :],
                              in_=deltad[b, :].rearrange("(c p) -> p c", p=128))
            qkv_all = [st.qkvp.tile([128, 2, H, 64], BF16, tag="qkv",
                                    name=f"qkv{b}_{xc}") for xc in range(NXC)]

            if stkA is None:
                stkA = {}
                for hp in (0, 1):
                    stkA.update(_fft_qk(nc, st, b, hp))
            if b == 0:
                # iFFT weights are first needed ~100us in; don't let their
                # 2 MiB of DMA delay the critical q/k/v lead-in loads
                nc.sync.dma_start(out=st.gre[:, :, :],
                                  in_=gred.rearrange("(c p) l -> p c l", p=128))
                nc.sync.dma_start(out=st.gim[:, :, :],
                                  in_=gimd.rearrange("(c p) l -> p c l", p=128))
            have_next = b + 1 < bl
            last = not have_next

            stkB_box = {}

            def fill_fftB(bb=b):
                for hp in (2, 3):
                    stkB_box.update(_fft_qk(nc, st, bb, hp))

            def fill_next(bb=b):
                # next batch's v-FFT and wave-0 q/k FFT fill the wave-1
                # sqrt/exp sections of this batch
                st.vf_next = _phase_v(nc, st, bb + 1)
                nxt = {}
                for hp in (0, 1):
                    nxt.update(_fft_qk(nc, st, bb + 1, hp))
                st.stkA_next = nxt

            def ichunk(lo, hi, p=pending):
                def go():
                    _ifft(nc, st, *p, range(lo, hi))
                return go

            st.vf_next = None
            st.stkA_next = None
            # wave-1 FFT fills wave-0's sqrt/exp sections; deferred
            # prev-batch iFFT chunks and next-batch v-FFT/q-k-FFT fill the
            # wave-1 sections and the final-batch tail.
            sqs0 = _wave_qk(nc, st, b, 0, stkA)
            _wave_sqrt(nc, st, sqs0)
            fill_fftB()
            ps0 = _wave_exp(nc, st, sqs0)
            if last and pending is not None:
                ichunk(0, 2)()
            _wave_av(nc, st, b, 0, ps0, vf, qkv_all)
            sqs1 = _wave_qk(nc, st, b, 1, stkB_box)
            _wave_sqrt(nc, st, sqs1)
            if last and pending is not None:
                ichunk(2, 5)()
            elif have_next:
                fill_next()
            elif pending is not None:
                _ifft(nc, st, *pending, range(0, NLC))
                pending = None
            ps1 = _wave_exp(nc, st, sqs1)
            if last and pending is not None:
                ichunk(5, NLC)()
                pending = None
            _wave_av(nc, st, b, 1, ps1, vf, qkv_all)
            stkA = st.stkA_next
            if have_next:
                pending = (b, tau_sb, delta_sb, qkv_all)
                vf = st.vf_next
            else:
                _ifft(nc, st, b, tau_sb, delta_sb, qkv_all, range(0, NLC))


def _ifft(nc, st, b, tau_sb, delta_sb, qkv_all, lcs_range, act_epi=False):
    for lc in lcs_range:
        lcs = slice(lc * 128, (lc + 1) * 128)
        ps_o = st.ps.tile([128, 512], F32, tag="ps", name=f"pso{b}_{lc}")
        for xc in range(NXC):
            nc.tensor.matmul(ps_o[:, 0:512],
                             st.gre[:, xc, lcs],
                             qkv_all[xc][:, 0, :, :].rearrange("p h e -> p (h e)"),
                             start=(xc == 0), stop=False)
            nc.tensor.matmul(ps_o[:, 0:512],
                             st.gim[:, xc, lcs],
                             qkv_all[xc][:, 1, :, :].rearrange("p h e -> p (h e)"),
                             start=False, stop=(xc == NXC - 1))
        out_t = st.ep.tile([128, 512], BF16, tag="outt", bufs=3,
                           name=f"out{b}_{lc}")
        if act_epi:
            # Identity(scale*x + bias) = tau*x + delta; Identity is in
            # every ACT table set, so this cannot thrash table loads
            nc.scalar.activation(out=out_t[:, :], in_=ps_o[:, 0:512],
                                 func=AF.Identity,
                                 bias=delta_sb[:, lc:lc + 1],
                                 scale=tau_sb[:, 0:1])
        else:
            nc.vector.tensor_scalar(out=out_t[:, :], in0=ps_o[:, 0:512],
                                    scalar1=tau_sb[:, 0:1],
                                    scalar2=delta_sb[:, lc:lc + 1],
                                    op0=st.OPS.mult, op1=st.OPS.add)
        nc.sync.dma_start(
            out=st.outd[b, lcs, :, :].rearrange("l h e -> l (h e)"),
            in_=out_t[:, :])


def _phase_v(nc, st, b):
    """v[b] load + transposed FFT -> (vf_av[yc], v512) in AV layout."""
    v_sb = st.io.tile([128, NLC, H * E], BF16, tag="vsb", bufs=2,
                      name=f"vsb{b}")
    nc.sync.dma_start(
        out=v_sb[:, :, :],
        in_=st.vd[b].rearrange("(c p) h e -> p c (h e)", p=128))

    vf_av = []
    for yc in range(NYC):
        t = st.vfp.tile([128, H, 132], BF16, tag="vfav", bufs=8,
                        name=f"vfav{b}_{yc}")
        vf_av.append(t)
    v512 = st.vfp.tile([1, H, 132], BF16, tag="v512", bufs=2, name=f"v512_{b}")

    for part, f_sb in ((0, st.fre), (1, st.fim)):
        for yc in range(NYC):
            ps = st.ps.tile([128, 512], F32, tag="ps", name=f"psv{b}_{part}_{yc}")
            for c in range(NLC):
                nc.tensor.matmul(ps[:, 0:512],
                                 f_sb[:, c, yc * 128:(yc + 1) * 128],
                                 v_sb[:, c, :],
                                 start=(c == 0), stop=(c == NLC - 1))
            nc.vector.tensor_copy(
                out=vf_av[yc][:, :, part * 64:(part + 1) * 64],
                in_=ps[:, 0:512].rearrange("p (h e) -> p h e", h=H))
    # vf_av[0] imag partition 0 actually holds vf Nyquist RE (smuggled
    # through fim col 0): save it as v512's real part, then zero it.
    nc.gpsimd.dma_start(out=v512[0:1, :, 0:64],
                        in_=vf_av[0][0:1, :, 64:128])
    nc.vector.memset(vf_av[0][0:1, :, 64:128], 0.0)
    for yc in range(NYC):
        nc.vector.memset(vf_av[yc][:, :, 128:129], 1.0)
    nc.vector.memset(v512[0:1, :, 64:128], 0.0)
    nc.vector.memset(v512[0:1, :, 128:129], 1.0)
    return vf_av, v512


def _fft_qk(nc, st, b, hp):
    """FFT of q & k for head pair hp -> stacked qstk/kstk/kswp/krag tiles."""
    q_hp = st.io.tile([128, NLC, 128], BF16, tag="qhp", bufs=2,
                      name=f"qhp{b}_{hp}")
    nc.sync.dma_start(
        out=q_hp[:, :, :],
        in_=st.qd[b, :, 2 * hp:2 * hp + 2, :].rearrange("(c p) h e -> p c (h e)", p=128))
    k_hp = st.io.tile([128, NLC, 128], BF16, tag="khp", bufs=2,
                      name=f"khp{b}_{hp}")
    nc.sync.dma_start(
        out=k_hp[:, :, :],
        in_=st.kd[b, :, 2 * hp:2 * hp + 2, :].rearrange("(c p) h e -> p c (h e)", p=128))

    out = {}
    ps_qr = st.ps.tile([128, 512], F32, tag="ps", name=f"psqr{b}_{hp}")
    ps_qi = st.ps.tile([128, 512], F32, tag="ps", name=f"psqi{b}_{hp}")
    for c in range(NLC):
        lhsT = q_hp[:, c, :]
        nc.tensor.matmul(ps_qr[:, 0:512], lhsT, st.fre[:, c, :],
                         start=(c == 0), stop=(c == NLC - 1))
        nc.tensor.matmul(ps_qi[:, 0:512], lhsT, st.fim[:, c, :],
                         start=(c == 0), stop=(c == NLC - 1))
    st_qr = st.stg.tile([128, X], BF16, tag="st", bufs=6, name=f"stqr{b}_{hp}")
    nc.vector.tensor_scalar_mul(out=st_qr[:, :], in0=ps_qr[:, 0:512],
                                scalar1=0.125)
    st_qi = st.stg.tile([128, X], BF16, tag="st", bufs=6, name=f"stqi{b}_{hp}")
    nc.vector.tensor_scalar_mul(out=st_qi[:, :], in0=ps_qi[:, 0:512],
                                scalar1=0.125)

    ps_kr = st.ps.tile([128, 512], F32, tag="ps", name=f"pskr{b}_{hp}")
    ps_ki = st.ps.tile([128, 512], F32, tag="ps", name=f"pski{b}_{hp}")
    for c in range(NLC):
        lhsT = k_hp[:, c, :]
        nc.tensor.matmul(ps_kr[:, 0:512], lhsT, st.fre[:, c, :],
                         start=(c == 0), stop=(c == NLC - 1))
        nc.tensor.matmul(ps_ki[:, 0:512], lhsT, st.fim[:, c, :],
                         start=(c == 0), stop=(c == NLC - 1))
    st_kr = st.stg.tile([128, X], BF16, tag="st", bufs=6, name=f"stkr{b}_{hp}")
    nc.vector.tensor_copy(out=st_kr[:, :], in_=ps_kr[:, 0:512])
    st_ki = st.stg.tile([128, X], BF16, tag="st", bufs=6, name=f"stki{b}_{hp}")
    nc.vector.tensor_copy(out=st_ki[:, :], in_=ps_ki[:, 0:512])
    st_kin = st.stg.tile([128, X], BF16, tag="st", bufs=6, name=f"stkin{b}_{hp}")
    nc.vector.tensor_scalar_mul(out=st_kin[:, :], in0=ps_ki[:, 0:512],
                                scalar1=-1.0)

    for phi in range(2):
        h = 2 * hp + phi
        rows = slice(64 * phi, 64 * phi + 64)
        # st_qi/st_ki col 0 hold the head's Nyquist RE value (smuggled);
        # zero the stacked copies' bin-0 imag after the row DMAs.
        qstk = st.stk.tile([128, X], BF16, tag="qstk", name=f"qstk{b}_{h}")
        nc.sync.dma_start(out=qstk[0:64, :], in_=st_qr[rows, :])
        nc.sync.dma_start(out=qstk[64:128, :], in_=st_qi[rows, :])
        nc.gpsimd.memset(qstk[64:128, 0:1], 0.0)
        kstk = st.stk.tile([128, X], BF16, tag="kstk", name=f"kstk{b}_{h}")
        nc.sync.dma_start(out=kstk[0:64, :], in_=st_kr[rows, :])
        nc.sync.dma_start(out=kstk[64:128, :], in_=st_ki[rows, :])
        nc.gpsimd.memset(kstk[64:128, 0:1], 0.0)
        kswp = st.stk.tile([128, X], BF16, tag="kswp", name=f"kswp{b}_{h}")
        nc.sync.dma_start(out=kswp[0:64, :], in_=st_kin[rows, :])
        nc.sync.dma_start(out=kswp[64:128, :], in_=st_kr[rows, :])
        nc.gpsimd.memset(kswp[0:64, 0:1], 0.0)
        krag = st.stk.tile([128, 2], BF16, tag="krag", name=f"krag{b}_{h}")
        nc.gpsimd.memset(krag[:, :], 0.0)
        nc.gpsimd.dma_start(out=krag[0:64, 0:1], in_=st_ki[rows, 0:1])
        nc.gpsimd.dma_start(out=krag[64:128, 1:2], in_=st_ki[rows, 0:1])
        out[h] = (qstk, kstk, kswp, krag)
    return out


def _qk_head(nc, st, b, h, stk, ragS, ragI, i):
    """QK matmuls + |qk|^2 squares for one head; rag row i of ragS/ragI."""
    qstk, kstk, kswp, krag = stk[h]
    s_h = st.sp.tile([128, NYC, X], F32, tag="s", name=f"s{b}_{h}")
    # ragged y=512 attention row first so the rag chain (squares, add,
    # sqrt, exp) completes early — AV's rank-1 matmul needs it
    ps_g = st.ps.tile([128, 512], F32, tag="ps", name=f"psg{b}_{h}")
    nc.tensor.matmul(ps_g[0:2, 0:512], krag[:, :], qstk[:, 0:512],
                     start=True, stop=True)
    rg2 = st.rag.tile([2, X], BF16, tag="rg2", name=f"rg2{b}_{h}")
    nc.vector.tensor_copy(out=rg2[:, :], in_=ps_g[0:2, 0:512])
    nc.gpsimd.dma_start(out=ragS[i:i + 1, :], in_=rg2[0:1, :])
    nc.gpsimd.dma_start(out=ragI[i:i + 1, :], in_=rg2[1:2, :])
    for yc in range(NYC):
        ycs = slice(yc * 128, (yc + 1) * 128)
        pq = st.pq.tile([128, 1024], F32, tag="pq", name=f"pq{b}_{h}_{yc}")
        nc.tensor.matmul(pq[:, 0:512], kstk[:, ycs], qstk[:, 0:512],
                         start=True, stop=True)
        nc.tensor.matmul(pq[:, 512:1024], kswp[:, ycs], qstk[:, 0:512],
                         start=True, stop=True)
        tsq = st.sq.tile([128, 1024], BF16, tag="tsq", name=f"tsq{b}_{h}_{yc}")
        nc.scalar.square(out=tsq[:, :], in_=pq[:, :])
        nc.vector.tensor_add(s_h[:, yc, :], tsq[:, 0:512], tsq[:, 512:1024])
    return s_h


def _av_head(nc, st, b, h, p_h, er_h, vf, qkv_all):
    vf_av, v512 = vf
    for xc in range(NXC):
        xcs = slice(xc * 128, (xc + 1) * 128)
        ps_av = st.ps.tile([128, 512], F32, tag="ps", name=f"psav{b}_{h}_{xc}")
        for yc in range(NYC):
            nc.tensor.matmul(ps_av[:, 0:129], p_h[:, yc, xcs],
                             vf_av[yc][:, h, 0:129],
                             start=(yc == 0), stop=False)
        # rank-1 y=512 term last so the group never waits on the rag chain
        nc.tensor.matmul(ps_av[:, 0:129], er_h[0:1, xcs],
                         v512[0:1, h, 0:129], start=False, stop=True)
        rc = st.ep.tile([128, 1], F32, tag="rc", bufs=4, name=f"rc{b}_{h}_{xc}")
        nc.vector.reciprocal(out=rc[:, :], in_=ps_av[:, 128:129])
        nc.vector.tensor_scalar_mul(
            out=qkv_all[xc][:, :, h, :],
            in0=ps_av[:, 0:128].rearrange("p (t e) -> p t e", t=2),
            scalar1=rc[:, 0:1])


def _wave_qk(nc, st, b, w, stk):
    """QK + squares for a wave's 4 heads, plus the rag chain up to ragM."""
    heads = [4 * w + i for i in range(4)]
    ragS = st.rag.tile([4, X], BF16, tag="ragS", name=f"ragS{b}_{w}")
    ragI = st.rag.tile([4, X], BF16, tag="ragI", name=f"ragI{b}_{w}")
    s_tiles = {}
    for i, h in enumerate(heads):
        s_tiles[h] = _qk_head(nc, st, b, h, stk, ragS, ragI, i)
    # rag chain on fast engines (DVE add): if its sqrt became ready only
    # mid-exp-section, the scheduler would thrash table loads
    nc.scalar.square(out=ragS[:, :], in_=ragS[:, :])
    nc.scalar.square(out=ragI[:, :], in_=ragI[:, :])
    ragM = st.rag.tile([4, X], F32, tag="ragM", name=f"ragM{b}_{w}")
    nc.vector.tensor_add(ragM[:, :], ragS[:, :], ragI[:, :])
    return heads, s_tiles, ragM


def _wave_sqrt(nc, st, sqs):
    heads, s_tiles, ragM = sqs
    nc.scalar.sqrt(out=ragM[:, :], in_=ragM[:, :])
    for h in heads:
        nc.scalar.sqrt(out=s_tiles[h][:, :, :], in_=s_tiles[h][:, :, :])


def _wave_exp(nc, st, sqs):
    heads, s_tiles, ragM = sqs
    b_w = heads[0] // 4
    # rag exp first so the er copies land early
    ragP = st.rag.tile([4, X], BF16, tag="ragP", name=f"ragP_{b_w}")
    nc.scalar.activation(out=ragP[:, :], in_=ragM[:, :], func=AF.Exp)
    er = {}
    for i, h in enumerate(heads):
        e_h = st.rag.tile([1, X], BF16, tag="er", bufs=6, name=f"er_{h}")
        nc.gpsimd.dma_start(out=e_h[0:1, :], in_=ragP[i:i + 1, :])
        er[h] = e_h
    p_tiles = {}
    for h in heads:
        p_h = st.pp.tile([128, NYC, X], BF16, tag="p", name=f"p_{h}")
        nc.scalar.activation(out=p_h[:, :, :], in_=s_tiles[h][:, :, :],
                             func=AF.Exp)
        p_tiles[h] = p_h
    return p_tiles, er


def _wave_av(nc, st, b, w, ps_in, vf, qkv_all):
    heads = [4 * w + i for i in range(4)]
    p_tiles, er = ps_in
    for h in heads:
        _av_head(nc, st, b, h, p_tiles[h], er[h], vf, qkv_all)


_BUILT = None
_CONSTS = None


def _get_built():
    global _BUILT, _CONSTS
    if _BUILT is None:
        _BUILT = build_module()
        _CONSTS = _consts()
    return _BUILT, _CONSTS


def kernel(q, k, v, mask, tau, delta):
    global LAST_RESULT
    nc, (fre, fim, gre, gim) = _get_built()
    q = np.ascontiguousarray(np.asarray(q, dtype=np.float32)).astype(ml_dtypes.bfloat16)
    k = np.ascontiguousarray(np.asarray(k, dtype=np.float32)).astype(ml_dtypes.bfloat16)
    v = np.ascontiguousarray(np.asarray(v, dtype=np.float32)).astype(ml_dtypes.bfloat16)
    tau = np.ascontiguousarray(np.asarray(tau, dtype=np.float32))
    delta = np.ascontiguousarray(np.asarray(delta, dtype=np.float32))

    in_maps = []
    for i in range(NCORES):
        sl = slice(i * BL, (i + 1) * BL)
        in_maps.append({
            "qd": np.ascontiguousarray(q[sl]),
            "kd": np.ascontiguousarray(k[sl]),
            "vd": np.ascontiguousarray(v[sl]),
            "taud": np.ascontiguousarray(tau[sl]),
            "deltad": np.ascontiguousarray(delta[sl]),
            "fred": fre, "fimd": fim,
            "gred": gre, "gimd": gim,
        })
    res = run_bass_kernel_spmd(nc, in_maps, core_ids=list(range(NCORES)))
    LAST_RESULT = res
    out = np.concatenate([res.results[i]["outd"] for i in range(NCORES)], axis=0)
    return out.astype(np.float32)
